# revision 18
# baseline (speedup 1.0000x reference)
"""Trainium2 Bass kernel for nn_AttentionWithMemory (local-window MHA block).

Sharding: data-parallel over batch - one batch element per NeuronCore (8 cores).
Per core: x_b [1024,1024] -> qkv in-proj -> 16-head local attention (window 32,
band +-16) -> out-proj -> out_b [1024,1024].

v4: 64-query-block attention with head-pair packing along the PE partition dim.
  For each (head-pair hp, 64-token block s) the two heads' 64-query slices are
  stacked into the 128 output partitions via a block-diagonal stationary qTb
  (head0 queries at cols 0:64 with head0's 64 feature rows, head1 at 64:128),
  against kTs [128 feat-pair rows, 96-key window] - keys of both heads live on
  their own feature rows, so one [128K,128M,96N] matmul yields both heads'
  scores for the block, one exp covers both heads with a per-partition rowsum,
  one [128,96] transpose gives P^T, and two K=96 matmuls (one per head's
  feature half, M-packed into one psum) give ctx^T. 33/96 of computed score
  columns are in-band vs 33/160 for 128-query tiles, and there are no wing
  matmuls/copies at all.
  Everything else as v2/v3: streamed input DMA, software-pipelined emission
  with dense projection chains as PE p-state filler, mask folded to
  s*0.125 + (-1e4), v-bias folded into the ctx evict.
"""

import os
import sys

sys.path.insert(0, "/opt/trn_rl_repo")

import numpy as np

B, S, D = 8, 1024, 1024
H, HD = 16, 64
P = 128
NT = S // P   # 8 token tiles (projection granularity)
NB = S // 64  # 16 query blocks (attention granularity)
KW = 96       # key window per 64-query block
N_CORES = 8

_CACHE = {}


def _kst(s):
    return 0 if s == 0 else 64 * s - 16


def _build_nc():
    import concourse.bacc as bacc
    import concourse.mybir as mybir
    import concourse.tile as tile
    from concourse.masks import make_identity

    dt = mybir.dt
    f32, bf16 = dt.float32, dt.bfloat16
    Act = mybir.ActivationFunctionType
    Alu = mybir.AluOpType

    nc = bacc.Bacc("TRN2", target_bir_lowering=False, debug=False,
                   num_devices=N_CORES)

    xt_d = nc.dram_tensor("xt", [D, S], bf16, kind="ExternalInput").ap()
    wi_d = nc.dram_tensor("w_int", [D, 3 * D], bf16, kind="ExternalInput").ap()
    wo_d = nc.dram_tensor("w_outt", [D, D], bf16, kind="ExternalInput").ap()
    bin_d = nc.dram_tensor("b_in_t", [P, 16], f32, kind="ExternalInput").ap()
    bvt_d = nc.dram_tensor("bvt", [P, NT], f32, kind="ExternalInput").ap()
    bo_d = nc.dram_tensor("bo_bc", [P, D], bf16, kind="ExternalInput").ap()
    mask_d = nc.dram_tensor("maskn", [P, NB, KW], bf16, kind="ExternalInput").ap()
    out_d = nc.dram_tensor("out", [S, D], f32, kind="ExternalOutput").ap()

    with tile.TileContext(nc) as tc:
        with (
            tc.tile_pool(name="const", bufs=1) as cpool,
            tc.tile_pool(name="acts", bufs=1) as apool,
            tc.tile_pool(name="psb", bufs=2) as wk,
            tc.tile_pool(name="ptsb", bufs=3) as ptpool,
            tc.tile_pool(name="lr", bufs=2) as lrpool,
            tc.tile_pool(name="outsb", bufs=1) as outpool,
            tc.tile_pool(name="ps_mm", bufs=2, space="PSUM") as ps_mm,
            tc.tile_pool(name="ps_sc", bufs=2, space="PSUM") as ps_sc,
            tc.tile_pool(name="ps_pt", bufs=2, space="PSUM") as ps_pt,
            tc.tile_pool(name="ps_cx", bufs=2, space="PSUM") as ps_cx,
        ):
            # ---- persistent SBUF tensors ----
            xt = [cpool.tile([P, S], bf16, tag=f"xt{i}", name=f"xt{i}") for i in range(NT)]
            wi = [cpool.tile([P, 3 * D], bf16, tag=f"wi{i}", name=f"wi{i}") for i in range(NT)]
            wo = [cpool.tile([P, D], bf16, tag=f"wo{i}", name=f"wo{i}") for i in range(NT)]
            bint = cpool.tile([P, 16], f32, tag="bint", name="bint")
            bvt = cpool.tile([P, NT], f32, tag="bvt", name="bvt")
            bo = cpool.tile([P, D], bf16, tag="bo", name="bo")
            mask = cpool.tile([P, NB, KW], bf16, tag="mask", name="mask")
            ident = cpool.tile([P, P], bf16, tag="ident", name="ident")

            # kTs[hp]: head 2hp+hh keys on feature rows hh*64:(hh+1)*64.
            # 16 zero tail cols so the s=15 window can read 96 wide.
            SK = S + 16
            kTs = [apool.tile([P, SK], bf16, tag=f"kTs{i}", name=f"kTs{i}")
                   for i in range(NT)]
            # qTb[hp]: per 64-block s, block-diagonal stationary [128, 128]:
            # cols 0:64 head0 queries (rows 0:64 live), cols 64:128 head1.
            qTb = [apool.tile([P, NB, P], bf16, tag=f"qTb{i}", name=f"qTb{i}")
                   for i in range(NT)]
            v = [apool.tile([P, D], bf16, tag=f"v{i}", name=f"v{i}") for i in range(NT)]
            # per-block key-window copies of v: rows 0:96 = tokens
            # [kst(s), kst(s)+96)
            vw = [apool.tile([P, D], bf16, tag=f"vw{s}", name=f"vw{s}")
                  for s in range(NB)]
            ctxT = [apool.tile([P, S], bf16, tag=f"ctxT{i}", name=f"ctxT{i}") for i in range(NT)]

            # ---- input DMA, in consumption order ----
            for i in range(NT):
                nc.sync.dma_start(out=xt[i], in_=xt_d[i * P:(i + 1) * P, :])
                nc.sync.dma_start(out=wi[i][:, 2 * D:3 * D],
                                  in_=wi_d[i * P:(i + 1) * P, 2 * D:3 * D])
            nc.sync.dma_start(out=bint, in_=bin_d)
            nc.sync.dma_start(out=bvt, in_=bvt_d)
            nc.sync.dma_start(out=mask, in_=mask_d)
            for i in range(NT):
                nc.sync.dma_start(out=wi[i][:, 0:2 * D],
                                  in_=wi_d[i * P:(i + 1) * P, 0:2 * D])
            nc.sync.dma_start(out=bo, in_=bo_d)
            for i in range(NT):
                nc.sync.dma_start(out=wo[i], in_=wo_d[i * P:(i + 1) * P, :])

            nc.gpsimd.memset(vw[NB - 1][64:96, :], 0.0)
            for hp in range(NT):
                nc.gpsimd.memset(kTs[hp][:, S:SK], 0.0)
                nc.gpsimd.memset(qTb[hp][64:128, :, 0:64], 0.0)
                nc.gpsimd.memset(qTb[hp][0:64, :, 64:128], 0.0)
            make_identity(nc, ident)

            # ---- projection emitters ----
            def emit_v(st, nh):
                ps = ps_mm.tile([P, 512], f32, tag="mm", name="mmps")
                for dc in range(NT):
                    nc.tensor.matmul(
                        ps,
                        lhsT=xt[dc][:, st * P:(st + 1) * P],
                        rhs=wi[dc][:, 2 * D + nh * 512: 2 * D + (nh + 1) * 512],
                        start=(dc == 0), stop=(dc == NT - 1),
                    )
                # v bias is folded into the ctx evict (per-feature there)
                nc.scalar.activation(v[st][:, nh * 512:(nh + 1) * 512], ps, Act.Copy)

            def emit_vw(s):
                # rows 0:96 <- tokens [kst(s), kst(s)+96)
                a = _kst(s)
                u = s // 2
                if s == 0:
                    nc.sync.dma_start(out=vw[0][0:96, :], in_=v[0][0:96, :])
                elif s % 2 == 0:
                    nc.sync.dma_start(out=vw[s][0:16, :], in_=v[u - 1][112:128, :])
                    nc.sync.dma_start(out=vw[s][16:96, :], in_=v[u][0:80, :])
                elif s < NB - 1:
                    nc.sync.dma_start(out=vw[s][0:80, :], in_=v[u][48:128, :])
                    nc.sync.dma_start(out=vw[s][80:96, :], in_=v[u + 1][0:16, :])
                else:
                    # s=15: tokens [944,1040): 80 real rows; rows 80:96 unused
                    # (their P^T rows are exp(-inf)=0)
                    nc.sync.dma_start(out=vw[s][0:80, :], in_=v[u][48:128, :])

            def emit_k(fc, nh):
                ps = ps_mm.tile([P, 512], f32, tag="mm", name="mmps")
                for dc in range(NT):
                    nc.tensor.matmul(
                        ps,
                        lhsT=wi[dc][:, D + fc * P: D + (fc + 1) * P],
                        rhs=xt[dc][:, nh * 512:(nh + 1) * 512],
                        start=(dc == 0), stop=(dc == NT - 1),
                    )
                for hh in range(2):
                    sl = slice(hh * HD, (hh + 1) * HD)
                    nc.vector.tensor_scalar(
                        out=kTs[fc][sl, nh * 512:(nh + 1) * 512],
                        in0=ps[sl, :], scalar1=bint[sl, 8 + fc:8 + fc + 1],
                        scalar2=None, op0=Alu.add,
                    )

            def emit_q(fc, nh):
                ps = ps_mm.tile([P, 8, 64], f32, tag="mm", name="mmps")
                for dc in range(NT):
                    nc.tensor.matmul(
                        ps,
                        lhsT=wi[dc][:, fc * P:(fc + 1) * P],
                        rhs=xt[dc][:, nh * 512:(nh + 1) * 512],
                        start=(dc == 0), stop=(dc == NT - 1),
                    )
                for hh in range(2):
                    sl = slice(hh * HD, (hh + 1) * HD)
                    nc.vector.tensor_scalar(
                        out=qTb[fc][sl, nh * 8:(nh + 1) * 8, hh * 64:(hh + 1) * 64],
                        in0=ps[sl, :, :], scalar1=bint[sl, fc:fc + 1],
                        scalar2=None, op0=Alu.add,
                    )

            def emit_op(st):
                o_sb = outpool.tile([P, D], f32, tag="o", name="o_sb")
                for nh in range(2):
                    ps = ps_mm.tile([P, 512], f32, tag="mm", name="mmps")
                    for fc in range(NT):
                        nc.tensor.matmul(
                            ps,
                            lhsT=ctxT[fc][:, st * P:(st + 1) * P],
                            rhs=wo[fc][:, nh * 512:(nh + 1) * 512],
                            start=(fc == 0), stop=(fc == NT - 1),
                        )
                    nc.vector.tensor_add(o_sb[:, nh * 512:(nh + 1) * 512], ps,
                                         bo[:, nh * 512:(nh + 1) * 512])
                nc.sync.dma_start(out=out_d[st * P:(st + 1) * P, :], in_=o_sb)

            # ---- attention units: (block s, head-pair hp) ----
            p_sbs = {}
            r_ts = {}

            def score_unit(s, hp):
                sp = ps_sc.tile([P, KW], f32, tag="sc", name="sps")
                nc.tensor.matmul(
                    sp,
                    lhsT=qTb[hp][:, s, :],
                    rhs=kTs[hp][:, _kst(s):_kst(s) + KW],
                    start=True, stop=True,
                )
                nc.vector.scalar_tensor_tensor(
                    out=sp, in0=sp, scalar=0.125, in1=mask[:, s, :],
                    op0=Alu.mult, op1=Alu.add)
                p_sb = wk.tile([P, KW], bf16, tag=f"p{hp}", name="p_sb")
                l_t = lrpool.tile([P, 1], f32, tag=f"l{hp}", name="l_t")
                r_t = lrpool.tile([P, 1], f32, tag=f"r{hp}", name="r_t")
                nc.scalar.activation(p_sb, sp, Act.Exp, accum_out=l_t)
                nc.vector.reciprocal(r_t, l_t)
                p_sbs[(s, hp)] = p_sb
                r_ts[(s, hp)] = r_t

            def ctx_unit(s, hp):
                p_sb = p_sbs.pop((s, hp))
                r_t = r_ts.pop((s, hp))
                nc.vector.tensor_scalar(
                    out=p_sb, in0=p_sb, scalar1=r_t, scalar2=None, op0=Alu.mult)
                tps = ps_pt.tile([KW, P], bf16, tag="pt", name="tps")
                nc.tensor.matmul(tps, lhsT=p_sb, rhs=ident,
                                 is_transpose=True, start=True, stop=True)
                pt_sb = ptpool.tile([KW, P], bf16, tag="ptm", name="pt_sb")
                nc.scalar.activation(pt_sb, tps, Act.Copy)
                # ctx^T for the pair: two K=96 matmuls M-packed into one psum
                cps = ps_cx.tile([P, 64], f32, tag="cx", name="ctxps")
                for hh in range(2):
                    nc.tensor.matmul(
                        cps[hh * HD:(hh + 1) * HD, :],
                        lhsT=vw[s][0:KW, hp * P + hh * HD: hp * P + (hh + 1) * HD],
                        rhs=pt_sb[:, hh * HD:(hh + 1) * HD],
                        start=True, stop=True,
                        skip_group_check=True,
                        tile_position=(0, hh * HD),
                    )
                nc.vector.tensor_scalar(
                    out=ctxT[hp][:, s * 64:(s + 1) * 64], in0=cps,
                    scalar1=bvt[:, hp:hp + 1], scalar2=None, op0=Alu.add,
                )

            # ---- emission schedule ----
            for st in range(4):
                for nh in range(2):
                    emit_v(st, nh)
            for s in range(1, 7):
                emit_vw(s)
            for fc in range(NT):
                emit_k(fc, 0)
            emit_vw(0)
            for fc in range(NT):
                emit_q(fc, 0)
                score_unit(0, fc)

            def F(fn, *a):
                return lambda: fn(*a)

            # fillers keyed by merged block index (2 blocks ~ 1 old tile)
            fillers = {
                0: [F(emit_k, 0, 1), F(emit_k, 1, 1), F(emit_k, 2, 1)],
                1: [F(emit_k, 3, 1), F(emit_q, 0, 1), F(emit_q, 1, 1)],
                2: [F(emit_k, 4, 1), F(emit_k, 5, 1), F(emit_k, 6, 1)],
                3: [F(emit_k, 7, 1), F(emit_q, 2, 1), F(emit_q, 3, 1)],
                4: [F(emit_q, 4, 1), F(emit_q, 5, 1), F(emit_q, 6, 1)],
                5: [F(emit_q, 7, 1), F(emit_v, 4, 0), F(emit_v, 4, 1)],
                6: [F(emit_v, 5, 0), F(emit_v, 5, 1)],
                7: [F(emit_op, 0)],
                8: [F(emit_v, 6, 0), F(emit_v, 6, 1)],
                9: [F(emit_op, 1)],
                10: [F(emit_v, 7, 0), F(emit_v, 7, 1)],
                11: [F(emit_op, 2)],
                12: [F(emit_op, 3)],
                13: [F(emit_op, 4)],
                14: [F(emit_op, 5)],
                15: [F(emit_op, 6)],
            }
            post = {5: [F(emit_vw, 7), F(emit_vw, 8)],
                    6: [F(emit_vw, 9), F(emit_vw, 10)],
                    8: [F(emit_vw, 11), F(emit_vw, 12)],
                    10: [F(emit_vw, 13), F(emit_vw, 14), F(emit_vw, 15)]}

            # final tile's out-proj pipelined chunk-wise into the last blocks
            op7_ps = [None, None]
            o7_sb = outpool.tile([P, D], f32, tag="o", name="o7_sb")

            def op7_chunk(fc):
                for nh in range(2):
                    if op7_ps[nh] is None:
                        op7_ps[nh] = ps_sc.tile([P, 512], f32, tag="sc",
                                                name="op7ps")
                    nc.tensor.matmul(
                        op7_ps[nh],
                        lhsT=ctxT[fc][:, 7 * P:8 * P],
                        rhs=wo[fc][:, nh * 512:(nh + 1) * 512],
                        start=(fc == 0), stop=(fc == NT - 1),
                    )

            for s in range(NB):
                fl = list(fillers[s])
                for hp in range(NT):
                    ctx_unit(s, hp)
                    if s == NB - 1 and hp % 2 == 1:
                        op7_chunk(hp // 2 * 2)
                        op7_chunk(hp // 2 * 2 + 1)
                    if s < NB - 1:
                        score_unit(s + 1, hp)
                    if fl and hp % 2 == 1:
                        fl.pop(0)()
                for fn in fl:
                    fn()
                for fn in post.get(s, []):
                    fn()
            for nh in range(2):
                nc.vector.tensor_add(o7_sb[:, nh * 512:(nh + 1) * 512],
                                     op7_ps[nh],
                                     bo[:, nh * 512:(nh + 1) * 512])
            nc.sync.dma_start(out=out_d[7 * P:8 * P, :], in_=o7_sb)

    nc.compile()
    return nc


def _get_nc():
    if "nc" not in _CACHE:
        _CACHE["nc"] = _build_nc()
    return _CACHE["nc"]


def _prep_inputs(x, w_in, b_in, w_out, b_out, mask):
    import ml_dtypes
    bf16 = ml_dtypes.bfloat16

    x = np.asarray(x, np.float32)
    w_in = np.asarray(w_in, np.float32)
    b_in = np.asarray(b_in, np.float32)
    w_out = np.asarray(w_out, np.float32)
    b_out = np.asarray(b_out, np.float32)
    mask = np.asarray(mask)

    w_int = np.ascontiguousarray(w_in.T).astype(bf16)          # [D, 3D]
    w_outt = np.ascontiguousarray(w_out.T).astype(bf16)        # [D, D]
    # q,k bias per-partition layout: col c (= global feature chunk), row p
    b_qk = b_in[:2 * D].reshape(16, P).T.astype(np.float32).copy()  # [128,16]
    # v bias, feature-major per head-pair chunk: col hp, rows = 128 features
    bvt = np.ascontiguousarray(b_in[2 * D:].reshape(NT, P).T).astype(np.float32)
    bo_bc = np.broadcast_to(b_out, (P, D)).astype(bf16)

    allowed = ~mask.astype(bool)
    # [P, NB, KW]: partition p = query 64*s + (p%64) (both heads identical)
    mneg = np.full((NB, 64, KW), -10000.0, np.float32)
    for s in range(NB):
        a = _kst(s)
        wt = min(KW, S - a)
        mneg[s, :, :wt] = np.where(
            allowed[s * 64:(s + 1) * 64, a:a + wt], 0.0, -10000.0)
    half = np.ascontiguousarray(mneg.transpose(1, 0, 2))       # [64, NB, KW]
    maskn = np.concatenate([half, half], axis=0).astype(bf16)  # [128, NB, KW]

    in_maps = []
    for b in range(B):
        xt = np.ascontiguousarray(x[b].T).astype(bf16)         # [D, S]
        in_maps.append({
            "xt": xt, "w_int": w_int, "w_outt": w_outt,
            "b_in_t": b_qk, "bvt": bvt, "bo_bc": bo_bc,
            "maskn": maskn,
        })
    return in_maps


def run(x, w_in, b_in, w_out, b_out, mask, trace=False):
    from concourse.bass_utils import run_bass_kernel_spmd
    nc = _get_nc()
    in_maps = _prep_inputs(x, w_in, b_in, w_out, b_out, mask)
    res = run_bass_kernel_spmd(nc, in_maps, list(range(N_CORES)), trace=trace)
    out = np.stack([np.asarray(res.results[b]["out"], np.float32)
                    for b in range(B)])
    return out, res


def kernel(x, w_in, b_in, w_out, b_out, mask):
    out, _ = run(x, w_in, b_in, w_out, b_out, mask)
    return out


# revision 20
# speedup vs baseline: 1.0776x; 1.0776x over previous
"""Trainium2 Bass kernel for nn_AttentionWithMemory (local-window MHA block).

Sharding: data-parallel over batch - one batch element per NeuronCore (8 cores).
Per core: x_b [1024,1024] -> qkv in-proj -> 16-head local attention (window 32,
band +-16) -> out-proj -> out_b [1024,1024].

v3 schedule (vs v1 baseline, ~302us -> ~202us):
  - input DMA streamed in compute-consumption order (xt+w_v chunks interleaved
    first, then w_qk, biases/mask between, w_out last) so the first projection
    chain starts ~10us in and the rest of the 10MB load hides under compute.
  - score matmuls pair-packed: one [128K,128M,320N] matmul per head-pair per
    query tile against a [128, 2, S+32] packed kT (zeros on the other head's
    feature rows), fp32 psum [128,2,160].
  - mask+scale fused into one scalar_tensor_tensor: s*0.125 + maskneg
    (maskneg = 0 allowed / -1e4 banned, broadcast over the head dim) - exp
    needs no bias and stale psum columns at the seq tail are killed by the
    mask instead of memsets.
  - wings are 32 wide (no zero-pad, no per-head memsets).
  - software pipeline: ctx/transpose phase of tile t is emitted interleaved
    with the score/exp phase of tile t+1, with dense projection chains
    (v/kq token-half-1, out-proj) injected between units to keep the PE
    p-state ramped (PE drops 2.4->1.2 GHz whenever it idles >~100ns).
  - psum->sbuf P^T copies on scalar/vector, v-bias folded into the ctx evict
    (ctx rows are features there, so bias is a per-partition scalar).
  - out tiles written as one [128,1024] DMA of full 4KB DRAM rows (half the
    descriptors); the last tile's out-proj is accumulated chunk-by-chunk
    inside the final attention tile so only its evict+DMA trail the compute.
"""

import os
import sys

sys.path.insert(0, "/opt/trn_rl_repo")

import numpy as np

B, S, D = 8, 1024, 1024
H, HD = 16, 64
P = 128
NT = S // P  # 8 query/token tiles
W = 160      # key slice width per query tile
N_CORES = 8

_CACHE = {}


def _build_nc():
    import concourse.bacc as bacc
    import concourse.mybir as mybir
    import concourse.tile as tile
    from concourse.masks import make_identity

    dt = mybir.dt
    f32, bf16 = dt.float32, dt.bfloat16
    Act = mybir.ActivationFunctionType
    Alu = mybir.AluOpType

    nc = bacc.Bacc("TRN2", target_bir_lowering=False, debug=False,
                   num_devices=N_CORES)

    xt_d = nc.dram_tensor("xt", [D, S], bf16, kind="ExternalInput").ap()
    wi_d = nc.dram_tensor("w_int", [D, 3 * D], bf16, kind="ExternalInput").ap()
    wo_d = nc.dram_tensor("w_outt", [D, D], bf16, kind="ExternalInput").ap()
    bin_d = nc.dram_tensor("b_in_t", [P, 16], f32, kind="ExternalInput").ap()
    bvt_d = nc.dram_tensor("bvt", [P, NT], f32, kind="ExternalInput").ap()
    bo_d = nc.dram_tensor("bo_bc", [P, D], f32, kind="ExternalInput").ap()
    mask_d = nc.dram_tensor("maskn", [P, NT, W], bf16, kind="ExternalInput").ap()
    out_d = nc.dram_tensor("out", [S, D], f32, kind="ExternalOutput").ap()

    with tile.TileContext(nc) as tc:
        with (
            tc.tile_pool(name="const", bufs=1) as cpool,
            tc.tile_pool(name="acts", bufs=1) as apool,
            tc.tile_pool(name="psb", bufs=2) as wk,
            tc.tile_pool(name="ptsb", bufs=3) as ptpool,
            tc.tile_pool(name="lr", bufs=2) as lrpool,
            tc.tile_pool(name="outsb", bufs=1) as outpool,
            tc.tile_pool(name="ps_mm", bufs=2, space="PSUM") as ps_mm,
            tc.tile_pool(name="ps_sc", bufs=2, space="PSUM") as ps_sc,
            tc.tile_pool(name="ps_pt", bufs=2, space="PSUM") as ps_pt,
            tc.tile_pool(name="ps_cx", bufs=2, space="PSUM") as ps_cx,
        ):
            # ---- persistent SBUF tensors ----
            xt = [cpool.tile([P, S], bf16, tag=f"xt{i}", name=f"xt{i}") for i in range(NT)]
            wi = [cpool.tile([P, 3 * D], bf16, tag=f"wi{i}", name=f"wi{i}") for i in range(NT)]
            wo = [cpool.tile([P, D], bf16, tag=f"wo{i}", name=f"wo{i}") for i in range(NT)]
            bint = cpool.tile([P, 16], f32, tag="bint", name="bint")
            bvt = cpool.tile([P, NT], f32, tag="bvt", name="bvt")
            bo = cpool.tile([P, D], f32, tag="bo", name="bo")
            mask = cpool.tile([P, NT, W], bf16, tag="mask", name="mask")
            ident = cpool.tile([P, P], bf16, tag="ident", name="ident")

            # kTp[hp] = packed pair: head 2hp+hh at rows hh*64:(hh+1)*64 of
            # [:, hh, :], zeros on the other 64 rows so K=128 matmuls work.
            # 32 zero cols of tail padding let every score matmul take the
            # full 160-wide window (tail cols are masked to -1e4 anyway).
            SP_ = S + 32
            kTp = [apool.tile([P, 2, SP_], bf16, tag=f"kTp{i}", name=f"kTp{i}")
                   for i in range(NT)]
            qT = [apool.tile([P, S], bf16, tag=f"qT{i}", name=f"qT{i}") for i in range(NT)]
            v = [apool.tile([P, D], bf16, tag=f"v{i}", name=f"v{i}") for i in range(NT)]
            voff = [None] + [apool.tile([P, D], bf16, tag=f"voff{j}", name=f"voff{j}")
                             for j in range(1, NT + 1)]
            ctxT = [apool.tile([P, S], bf16, tag=f"ctxT{i}", name=f"ctxT{i}") for i in range(NT)]

            # ---- input DMA, in consumption order (first matmul needs
            # xt[0]+wi_v[0]; mask/bo are only needed tens of us in) ----
            for i in range(NT):
                nc.sync.dma_start(out=xt[i], in_=xt_d[i * P:(i + 1) * P, :])
                nc.sync.dma_start(out=wi[i][:, 2 * D:3 * D],
                                  in_=wi_d[i * P:(i + 1) * P, 2 * D:3 * D])
            nc.sync.dma_start(out=bint, in_=bin_d)
            nc.sync.dma_start(out=bvt, in_=bvt_d)
            nc.sync.dma_start(out=mask, in_=mask_d)
            for i in range(NT):
                nc.sync.dma_start(out=wi[i][:, 0:2 * D],
                                  in_=wi_d[i * P:(i + 1) * P, 0:2 * D])
            nc.sync.dma_start(out=bo, in_=bo_d)
            for i in range(NT):
                nc.sync.dma_start(out=wo[i], in_=wo_d[i * P:(i + 1) * P, :])

            for hp in range(NT):
                for hh in range(2):
                    nc.gpsimd.memset(kTp[hp][(1 - hh) * 64:(2 - hh) * 64, hh, :], 0.0)
                    nc.gpsimd.memset(kTp[hp][hh * 64:(hh + 1) * 64, hh, S:SP_], 0.0)
            make_identity(nc, ident)

            # ---- projection emitters ----
            def emit_v(st, nh):
                ps = ps_mm.tile([P, 512], f32, tag="mm", name="mmps")
                for dc in range(NT):
                    nc.tensor.matmul(
                        ps,
                        lhsT=xt[dc][:, st * P:(st + 1) * P],
                        rhs=wi[dc][:, 2 * D + nh * 512: 2 * D + (nh + 1) * 512],
                        start=(dc == 0), stop=(dc == NT - 1),
                    )
                # v bias is folded into the ctx evict (per-feature there)
                nc.scalar.activation(v[st][:, nh * 512:(nh + 1) * 512], ps, Act.Copy)

            def emit_voff(j):
                if j < NT:
                    nc.sync.dma_start(out=voff[j][0:16, :], in_=v[j - 1][112:128, :])
                    nc.sync.dma_start(out=voff[j][16:128, :], in_=v[j][0:112, :])
                else:
                    nc.vector.memset(voff[NT][0:32, :], 0.0)
                    nc.sync.dma_start(out=voff[NT][0:16, :], in_=v[NT - 1][112:128, :])

            def emit_k(fc, nh):
                ps = ps_mm.tile([P, 512], f32, tag="mm", name="mmps")
                for dc in range(NT):
                    nc.tensor.matmul(
                        ps,
                        lhsT=wi[dc][:, D + fc * P: D + (fc + 1) * P],
                        rhs=xt[dc][:, nh * 512:(nh + 1) * 512],
                        start=(dc == 0), stop=(dc == NT - 1),
                    )
                for hh in range(2):
                    sl = slice(hh * HD, (hh + 1) * HD)
                    nc.vector.tensor_scalar(
                        out=kTp[fc][sl, hh, nh * 512:(nh + 1) * 512],
                        in0=ps[sl, :], scalar1=bint[sl, 8 + fc:8 + fc + 1],
                        scalar2=None, op0=Alu.add,
                    )

            def emit_q(fc, nh):
                ps = ps_mm.tile([P, 512], f32, tag="mm", name="mmps")
                for dc in range(NT):
                    nc.tensor.matmul(
                        ps,
                        lhsT=wi[dc][:, fc * P:(fc + 1) * P],
                        rhs=xt[dc][:, nh * 512:(nh + 1) * 512],
                        start=(dc == 0), stop=(dc == NT - 1),
                    )
                nc.vector.tensor_scalar(
                    out=qT[fc][:, nh * 512:(nh + 1) * 512],
                    in0=ps, scalar1=bint[:, fc:fc + 1], scalar2=None, op0=Alu.add,
                )

            def emit_op(st):
                # both 512-col halves, evicted into one [128,1024] sbuf tile
                # and written as one DMA of full 4KB DRAM rows (half the
                # descriptors of two 2KB-row writes)
                o_sb = outpool.tile([P, D], f32, tag="o", name="o_sb")
                for nh in range(2):
                    ps = ps_mm.tile([P, 512], f32, tag="mm", name="mmps")
                    for fc in range(NT):
                        nc.tensor.matmul(
                            ps,
                            lhsT=ctxT[fc][:, st * P:(st + 1) * P],
                            rhs=wo[fc][:, nh * 512:(nh + 1) * 512],
                            start=(fc == 0), stop=(fc == NT - 1),
                        )
                    nc.vector.tensor_add(o_sb[:, nh * 512:(nh + 1) * 512], ps,
                                         bo[:, nh * 512:(nh + 1) * 512])
                nc.sync.dma_start(out=out_d[st * P:(st + 1) * P, :], in_=o_sb)

            # ---- attention units ----
            # per (tile, head-pair) state carried from score phase to ctx phase
            p_sbs = {}   # (t, hp) -> p_sb tile
            r_ts = {}    # (t, hp) -> r tile

            def score_unit(t, hp):
                kst = 0 if t == 0 else t * P - 16
                sp = ps_sc.tile([P, 2, W], f32, tag="sc", name="sps")
                nc.tensor.matmul(
                    sp,
                    lhsT=qT[hp][:, t * P:(t + 1) * P],
                    rhs=kTp[hp][:, :, kst:kst + W],
                    start=True, stop=True,
                )
                # s*0.125 + maskneg  (maskneg: 0 allowed / -1e4 banned; also
                # kills stale psum cols beyond w_t on the last tile)
                nc.vector.scalar_tensor_tensor(
                    out=sp, in0=sp, scalar=0.125,
                    in1=mask[:, t, :].unsqueeze(1).broadcast_to((P, 2, W)),
                    op0=Alu.mult, op1=Alu.add)
                p_sb = wk.tile([P, 2, W], bf16, tag=f"p{hp}", name="p_sb")
                l_t = lrpool.tile([P, 2], f32, tag=f"l{hp}", name="l_t")
                r_t = lrpool.tile([P, 2], f32, tag=f"r{hp}", name="r_t")
                for hh in range(2):
                    nc.scalar.activation(p_sb[:, hh, :], sp[:, hh, :], Act.Exp,
                                         accum_out=l_t[:, hh:hh + 1])
                nc.vector.reciprocal(r_t, l_t)
                p_sbs[(t, hp)] = p_sb
                r_ts[(t, hp)] = r_t

            def ctx_unit(t, hp):
                p_sb = p_sbs.pop((t, hp))
                r_t = r_ts.pop((t, hp))
                vm = v[0] if t == 0 else voff[t]
                vc = v[1] if t == 0 else voff[t + 1]
                # scale P rows by 1/l (per-partition scalar), still bf16
                for hh in range(2):
                    nc.vector.tensor_scalar(
                        out=p_sb[:, hh, :], in0=p_sb[:, hh, :],
                        scalar1=r_t[:, hh:hh + 1], scalar2=None, op0=Alu.mult,
                    )
                # transposes: mains [128,128] x2, wings [32,128] x2 (no pad),
                # all packed into one psum tile (cols 0:256 mains, 256:512
                # wings with only rows 0:32 meaningful)
                mps = ps_pt.tile([P, 4 * P], bf16, tag="pt", name="mps")
                for hh in range(2):
                    nc.tensor.matmul(
                        mps[:, hh * P:(hh + 1) * P], lhsT=p_sb[:, hh, 0:P],
                        rhs=ident, is_transpose=True, start=True, stop=True,
                    )
                for hh in range(2):
                    nc.tensor.matmul(
                        mps[0:32, (2 + hh) * P:(3 + hh) * P],
                        lhsT=p_sb[:, hh, P:W],
                        rhs=ident, is_transpose=True, start=True, stop=True,
                    )
                pt_sb = ptpool.tile([P, 4 * P], bf16, tag="ptm", name="pt_sb")
                nc.scalar.activation(pt_sb[:, 0:2 * P], mps[:, 0:2 * P], Act.Copy)
                nc.vector.tensor_copy(pt_sb[0:32, 2 * P:4 * P],
                                      mps[0:32, 2 * P:4 * P])
                # ctx^T [hd, q]: per head main+wing accumulation, heads
                # col-packed into one psum bank
                cps = ps_cx.tile([P, P], f32, tag="cx", name="ctxps")
                for hh in range(2):
                    h = hp * 2 + hh
                    nc.tensor.matmul(
                        cps[hh * HD:(hh + 1) * HD, :],
                        lhsT=vm[:, h * HD:(h + 1) * HD],
                        rhs=pt_sb[:, hh * P:(hh + 1) * P],
                        start=True, stop=False,
                        skip_group_check=True,
                        tile_position=(0, hh * HD),
                    )
                for hh in range(2):
                    h = hp * 2 + hh
                    nc.tensor.matmul(
                        cps[hh * HD:(hh + 1) * HD, :],
                        lhsT=vc[0:32, h * HD:(h + 1) * HD],
                        rhs=pt_sb[0:32, (2 + hh) * P:(3 + hh) * P],
                        start=False, stop=True,
                        skip_group_check=True,
                        tile_position=(0, hh * HD),
                    )
                # evict + v-bias (per-feature = per-partition here) + cast
                nc.vector.tensor_scalar(
                    out=ctxT[hp][:, t * P:(t + 1) * P], in0=cps,
                    scalar1=bvt[:, hp:hp + 1], scalar2=None, op0=Alu.add,
                )

            # ---- emission schedule ----
            # pre-phase: v(0..3), k(nh=0), q(nh=0) with tile-0 scores injected
            for st in range(4):
                for nh in range(2):
                    emit_v(st, nh)
            for j in range(1, 4):
                emit_voff(j)
            for fc in range(NT):
                emit_k(fc, 0)
            for fc in range(NT):
                emit_q(fc, 0)
                score_unit(0, fc)

            def F(fn, *a):
                return lambda: fn(*a)

            fillers = {
                0: [F(emit_k, 0, 1), F(emit_k, 1, 1), F(emit_k, 2, 1),
                    F(emit_k, 3, 1), F(emit_q, 0, 1), F(emit_q, 1, 1)],
                1: [F(emit_k, 4, 1), F(emit_k, 5, 1), F(emit_k, 6, 1),
                    F(emit_k, 7, 1), F(emit_q, 2, 1), F(emit_q, 3, 1)],
                2: [F(emit_q, 4, 1), F(emit_q, 5, 1), F(emit_q, 6, 1),
                    F(emit_q, 7, 1), F(emit_v, 4, 0), F(emit_v, 4, 1)],
                3: [F(emit_v, 5, 0), F(emit_v, 5, 1), F(emit_op, 0)],
                4: [F(emit_v, 6, 0), F(emit_v, 6, 1), F(emit_op, 1)],
                5: [F(emit_v, 7, 0), F(emit_v, 7, 1), F(emit_op, 2)],
                6: [F(emit_op, 3), F(emit_op, 4)],
                7: [F(emit_op, 5), F(emit_op, 6)],
            }
            post = {2: [F(emit_voff, 4)], 3: [F(emit_voff, 5)],
                    4: [F(emit_voff, 6)], 5: [F(emit_voff, 7), F(emit_voff, 8)]}

            # final tile's out-proj is pipelined chunk-wise into the last
            # merged tile: chunk fc of both halves right after ctx_unit(7,fc)
            op7_ps = [None, None]
            o7_sb = outpool.tile([P, D], f32, tag="o7", name="o7_sb")

            def op7_chunk(fc):
                for nh in range(2):
                    if op7_ps[nh] is None:
                        op7_ps[nh] = ps_sc.tile([P, 512], f32, tag="sc",
                                                name="op7ps")
                    nc.tensor.matmul(
                        op7_ps[nh],
                        lhsT=ctxT[fc][:, 7 * P:8 * P],
                        rhs=wo[fc][:, nh * 512:(nh + 1) * 512],
                        start=(fc == 0), stop=(fc == NT - 1),
                    )

            for t in range(NT):
                fl = list(fillers[t])
                for hp in range(NT):
                    ctx_unit(t, hp)
                    if t == NT - 1:
                        op7_chunk(hp)
                    if t < NT - 1:
                        score_unit(t + 1, hp)
                    if fl:
                        fl.pop(0)()
                for fn in fl:
                    fn()
                for fn in post.get(t, []):
                    fn()
            for nh in range(2):
                nc.vector.tensor_add(o7_sb[:, nh * 512:(nh + 1) * 512],
                                     op7_ps[nh],
                                     bo[:, nh * 512:(nh + 1) * 512])
            nc.sync.dma_start(out=out_d[7 * P:8 * P, :], in_=o7_sb)

    nc.compile()
    return nc


def _get_nc():
    if "nc" not in _CACHE:
        _CACHE["nc"] = _build_nc()
    return _CACHE["nc"]


def _prep_inputs(x, w_in, b_in, w_out, b_out, mask):
    import ml_dtypes
    bf16 = ml_dtypes.bfloat16

    x = np.asarray(x, np.float32)
    w_in = np.asarray(w_in, np.float32)
    b_in = np.asarray(b_in, np.float32)
    w_out = np.asarray(w_out, np.float32)
    b_out = np.asarray(b_out, np.float32)
    mask = np.asarray(mask)

    w_int = np.ascontiguousarray(w_in.T).astype(bf16)          # [D, 3D]
    w_outt = np.ascontiguousarray(w_out.T).astype(bf16)        # [D, D]
    # q,k bias per-partition layout: col c (= global feature chunk), row p
    b_qk = b_in[:2 * D].reshape(16, P).T.astype(np.float32).copy()  # [128,16]
    # v bias, feature-major per head-pair chunk: col hp, rows = 128 features
    bvt = np.ascontiguousarray(b_in[2 * D:].reshape(NT, P).T).astype(np.float32)
    bo_bc = np.broadcast_to(b_out.astype(np.float32), (P, D)).copy()

    allowed = ~mask.astype(bool)
    mneg = np.full((NT, P, W), -10000.0, np.float32)
    for t in range(NT):
        kst = 0 if t == 0 else t * P - 16
        wt = min(W, S - kst)
        mneg[t, :, :wt] = np.where(allowed[t * P:(t + 1) * P, kst:kst + wt],
                                   0.0, -10000.0)
    maskn = np.ascontiguousarray(mneg.transpose(1, 0, 2)).astype(bf16)

    in_maps = []
    for b in range(B):
        xt = np.ascontiguousarray(x[b].T).astype(bf16)         # [D, S]
        in_maps.append({
            "xt": xt, "w_int": w_int, "w_outt": w_outt,
            "b_in_t": b_qk, "bvt": bvt, "bo_bc": bo_bc,
            "maskn": maskn,
        })
    return in_maps


def run(x, w_in, b_in, w_out, b_out, mask, trace=False):
    from concourse.bass_utils import run_bass_kernel_spmd
    nc = _get_nc()
    in_maps = _prep_inputs(x, w_in, b_in, w_out, b_out, mask)
    res = run_bass_kernel_spmd(nc, in_maps, list(range(N_CORES)), trace=trace)
    out = np.stack([np.asarray(res.results[b]["out"], np.float32)
                    for b in range(B)])
    return out, res


def kernel(x, w_in, b_in, w_out, b_out, mask):
    out, _ = run(x, w_in, b_in, w_out, b_out, mask)
    return out
